# revision 33
# baseline (speedup 1.0000x reference)
"""BottleneckAttn Trainium2 kernel (bf16 PE pipeline).

Full inputs -> full output. 8-way head-parallel sharding, one (batch, head)
pair per NeuronCore. Per core, a fused transposed-attention kernel:

  attn^T[m, n] = sum_d k[d,m] q'[d,n] + XH^T[h'(m), n] + XW^T[w'(m), n]
  P^T = exp(attn^T)            (no row-max subtraction; logits are O(10))
  unnorm^T[dv, n] = sum_m v[m, dv] P^T[m, n];  S[n] = sum_m P^T[m, n]

with q' = SCALE*q folded into the q projection weights and rel tables
pre-scaled by 1/SCALE (exact powers of two). XH is injected into the logits
through 64 extra contraction rows (E_h selector stacked under k in the
stationary operand); XW through a second accumulating matmul against the
constant [I|I] selector. The softmax denominator S comes out as a 65th row
of the AV matmul (ones column in v^T); the final division by S happens on
the host (0.1% of the FLOPs) so the device never switches activation
tables away from Exp.

All attention matmuls run in bf16 (fp32 PSUM accumulation): on TRN2 bf16
streams at the same 1 column/cycle as fp32r but enables the compiler's
fast-weight-load path, which dominated the fp32r version. The q'/k
projection uses two M=128 stationaries [wq'|wk] and [wk|wq'] so both q'
(from A) and k (from B) land on PSUM partitions 0:64, matching their SBUF
row destinations without cross-partition moves. v^T is padded to a
128-column stationary for the same reason (cols 65:128 multiply into PSUM
partitions that are never read). The v^T projection and the XH blocks not
needed by the first n-block are interleaved into the first attention
n-block's instruction stream, where the PE and DVE have slack.
"""

import sys

if "/opt/trn_rl_repo" not in sys.path:
    sys.path.insert(0, "/opt/trn_rl_repo")

import ml_dtypes
import numpy as np

import concourse.bass as bass
import concourse.tile as tile
from concourse import bacc, mybir
from concourse.bass_utils import run_bass_kernel_spmd

B, C, H, W = 2, 256, 64, 64
NH, D = 4, 64
HW = H * W   # 4096
NMC = 32     # m chunks of 128
NBQ = 4      # n blocks of 1024
FP32 = mybir.dt.float32
BF16 = mybir.dt.bfloat16
AF = mybir.ActivationFunctionType
NPBF = ml_dtypes.bfloat16

_prog = None


def _build():
    nc = bacc.Bacc("TRN2", target_bir_lowering=False, debug=False)

    x_d = nc.dram_tensor("x", [2, 128, HW], BF16, kind="ExternalInput").ap()
    wa_d = nc.dram_tensor("wa", [2, 128, 128], BF16, kind="ExternalInput").ap()
    wb_d = nc.dram_tensor("wb", [2, 128, 128], BF16, kind="ExternalInput").ap()
    wv_d = nc.dram_tensor("wv", [2, 128, D], BF16, kind="ExternalInput").ap()
    hrel_d = nc.dram_tensor("hrel", [64, 127], BF16, kind="ExternalInput").ap()
    wrel_d = nc.dram_tensor("wrel", [64, 127], BF16, kind="ExternalInput").ap()
    eh_d = nc.dram_tensor("eh", [64, HW], BF16, kind="ExternalInput").ap()
    ew_d = nc.dram_tensor("ew", [64, 128], BF16, kind="ExternalInput").ap()
    onesv_d = nc.dram_tensor("onesv", [128, NMC], BF16, kind="ExternalInput").ap()
    out_d = nc.dram_tensor("out", [D + 1, HW], FP32, kind="ExternalOutput").ap()

    with tile.TileContext(nc) as tc:
        with (
            tc.tile_pool(name="const", bufs=1) as constp,
            tc.tile_pool(name="big", bufs=1) as bigp,
            tc.tile_pool(name="ptp", bufs=4) as ptp,
            tc.tile_pool(name="unn", bufs=2) as unnp,
            tc.tile_pool(name="ps", bufs=2, space="PSUM") as psp,
            tc.tile_pool(name="av_ps", bufs=1, space="PSUM") as avp,
        ):
            # ---------------- input tiles ----------------
            x_sb = bigp.tile([128, 2, HW], BF16)       # x[b]: (c, n), c = t*128+p
            wa_sb = constp.tile([128, 2, 128], BF16)   # [wq' | wk]
            wb_sb = constp.tile([128, 2, 128], BF16)   # [wk | wq']
            wv_sb = constp.tile([128, 2, D], BF16)
            hrel_sb = constp.tile([64, 127], BF16)     # height_rel^T / SCALE
            wrel_sb = constp.tile([64, 127], BF16)
            ew_sb = constp.tile([64, 128], BF16)       # [I64 | I64]
            # mm1 stationary: rows 0:64 = k, rows 64:128 = E_h; free = (mc, i)
            keh = bigp.tile([128, NMC, 128], BF16)
            # mm1 moving: rows 0:64 = q', rows 64:128 = XH^T; free = (h, w)
            rhs1 = bigp.tile([128, 64, 64], BF16)
            xw_t = bigp.tile([64, 64, 64], BF16)       # XW^T, free = (g, w)
            v_t = bigp.tile([128, NMC, 128], BF16)     # v^T + ones col + 63 junk

            for t in range(2):
                nc.sync.dma_start(out=wa_sb[:, t, :], in_=wa_d[t])
                nc.sync.dma_start(out=wb_sb[:, t, :], in_=wb_d[t])
                nc.sync.dma_start(out=wv_sb[:, t, :], in_=wv_d[t])
            nc.sync.dma_start(out=hrel_sb[:, :], in_=hrel_d[:, :])
            nc.sync.dma_start(out=wrel_sb[:, :], in_=wrel_d[:, :])
            for cb in range(8):
                xsl = slice(cb * 512, (cb + 1) * 512)
                for t in range(2):
                    nc.sync.dma_start(out=x_sb[:, t, xsl], in_=x_d[t, :, xsl])
            nc.sync.dma_start(out=keh[64:128, :, :], in_=eh_d[:, :])
            nc.sync.dma_start(out=ew_sb[:, :], in_=ew_d[:, :])
            nc.sync.dma_start(out=v_t[:, :, D], in_=onesv_d[:, :])

            # ---------------- phase 1: q', k projections ----------------
            # A = [wq'|wk], B = [wk|wq']: both q' (from A) and k (from B)
            # come out on PSUM partitions 0:64.
            # pre-phase psum tiles alternate between the "sm" and (otherwise
            # idle until phase 5) "pp"/"av" rings for 4-deep pipelining
            pre_n = 0

            def pre_ps(shape, name):
                nonlocal pre_n
                pre_n += 1
                if pre_n % 2 == 0:
                    return psp.tile(shape, FP32, name=name, tag="sm")
                elif pre_n % 4 == 1:
                    return psp.tile(shape, FP32, name=name, tag="pp")
                else:
                    return avp.tile(shape, FP32, name=name, tag="av")

            cp_n = 0

            def copy_alt(dst, src):
                # alternate copies between DVE and ACT (both idle pre-ph5)
                nonlocal cp_n
                cp_n += 1
                if cp_n % 2 == 0:
                    nc.scalar.copy(dst, src)
                else:
                    nc.vector.tensor_copy(dst, src)

            def vt_pair(mc):
                # v^T chunks mc, mc+1: stationary = x chunk (FWL), out = m
                psv = pre_ps([128, 2, D], "psv")
                for half in range(2):
                    msl = slice((mc + half) * 128, (mc + half + 1) * 128)
                    for t in range(2):
                        nc.tensor.matmul(
                            psv[:, half, :], x_sb[:, t, msl], wv_sb[:, t, :],
                            start=(t == 0), stop=(t == 1),
                        )
                copy_alt(v_t[:, mc:mc + 2, 0:D], psv[:, :, :])

            def xh_pair(r, on_act):
                # XH^T blocks r, r+1 -> rhs1[64:128, r:r+2, :]
                psh = pre_ps([128, 2, 64], "psh")
                for half in range(2):
                    nc.tensor.matmul(
                        psh[64:128, half, :],
                        hrel_sb[:, 63 - r - half:127 - r - half],
                        rhs1[0:64, r + half, :], start=True, stop=True,
                    )
                if on_act:
                    nc.scalar.copy(rhs1[64:128, r:r + 2, :], psh[64:128, :, :])
                else:
                    nc.vector.tensor_copy(
                        rhs1[64:128, r:r + 2, :], psh[64:128, :, :]
                    )

            ph1_emit = []

            for nb in range(8):
                sl = slice(nb * 512, (nb + 1) * 512)
                psa = pre_ps([128, 512], "psa")
                for t in range(2):
                    nc.tensor.matmul(
                        psa[:, :], wa_sb[:, t, :], x_sb[:, t, sl],
                        start=(t == 0), stop=(t == 1),
                    )
                nc.vector.tensor_copy(
                    rhs1[0:64, 8 * nb:8 * (nb + 1), :], psa[0:64, :]
                )
                psb = pre_ps([128, 512], "psb")
                for t in range(2):
                    nc.tensor.matmul(
                        psb[:, :], wb_sb[:, t, :], x_sb[:, t, sl],
                        start=(t == 0), stop=(t == 1),
                    )
                nc.scalar.copy(
                    keh[0:64, 4 * nb:4 * (nb + 1), :], psb[0:64, :]
                )
                # XH blocks for the previous projection block fill the PE's
                # x-DMA wait slack (block r only needs q' columns r//8)
                for fn in ph1_emit:
                    fn()
                ph1_emit = [
                    (lambda r=8 * nb + 2 * p: xh_pair(r, on_act=(r % 4 == 0)))
                    for p in range(4)
                ]
            for fn in ph1_emit:
                fn()

            # ---------- phase 4: XW^T, with v^T chunks interleaved -------
            for r in range(0, 64, 2):
                psw = pre_ps([64, 2, 64], "psw")
                for half in range(2):
                    nc.tensor.matmul(
                        psw[:, half, :],
                        wrel_sb[:, 63 - r - half:127 - r - half],
                        rhs1[0:64, :, r + half], start=True, stop=True,
                    )
                copy_alt(
                    xw_t[:, :, r:r + 2], psw[:, :, :].transpose([0, 2, 1])
                )
                if r % 8 == 2:
                    vt_pair((r // 8) * 4)
                    vt_pair((r // 8) * 4 + 2)

            # ---------------- phase 5: attention main loop ----------------
            SKEW = 3
            for nbq in range(NBQ):
                av = avp.tile([128, 1024], FP32, name="av", tag="av")
                pts = {}
                for mc in range(NMC + SKEW):
                    if mc < NMC:
                        pp = psp.tile([128, 1024], FP32, name="pp", tag="pp")
                        for i in range(2):
                            nc.tensor.matmul(
                                pp[:, 512 * i:512 * (i + 1)],
                                keh[:, mc, :],
                                rhs1[:, 16 * nbq + 8 * i:16 * nbq + 8 * (i + 1), :],
                                start=True, stop=False,
                            )
                    if mc >= SKEW:
                        j = mc - SKEW
                        ptj = pts.pop(j)
                        for i in range(2):
                            nc.tensor.matmul(
                                av[:, 512 * i:512 * (i + 1)], v_t[:, j, :],
                                ptj[:, 512 * i:512 * (i + 1)],
                                start=(j == 0), stop=(j == NMC - 1),
                            )
                    if mc < NMC:
                        for i in range(2):
                            nc.tensor.matmul(
                                pp[:, 512 * i:512 * (i + 1)], ew_sb[:, :],
                                xw_t[:, 16 * nbq + 8 * i:16 * nbq + 8 * (i + 1), :],
                                start=False, stop=True,
                            )
                        pt = ptp.tile([128, 1024], BF16, name="pt")
                        nc.scalar.activation(pt[:, :], pp[:, :], AF.Exp)
                        pts[mc] = pt
                # unnormalized out^T rows 0:64 + S row; stage through SBUF
                for i in range(2):
                    unn = unnp.tile([65, 512], FP32, name="unn")
                    nc.vector.tensor_copy(
                        unn[:, :], av[0:65, 512 * i:512 * (i + 1)]
                    )
                    nc.sync.dma_start(
                        out=out_d[:, nbq * 1024 + 512 * i:
                                  nbq * 1024 + 512 * (i + 1)],
                        in_=unn[:, :],
                    )

    nc.finalize()
    return nc


def _get_program():
    global _prog
    if _prog is None:
        _prog = _build()
    return _prog


def _make_in_maps(x, qkv_w, height_rel, width_rel):
    x = np.ascontiguousarray(np.asarray(x, dtype=np.float32))
    qkv_w = np.ascontiguousarray(np.asarray(qkv_w, dtype=np.float32))
    height_rel = np.asarray(height_rel, dtype=np.float32)
    width_rel = np.asarray(width_rel, dtype=np.float32)

    # exact power-of-two rescale: q' = q/8 folded into wq, rel tables * 8
    hrel_t = np.ascontiguousarray((height_rel * np.float32(8.0)).T).astype(NPBF)
    wrel_t = np.ascontiguousarray((width_rel * np.float32(8.0)).T).astype(NPBF)

    eh = np.zeros((64, HW), dtype=NPBF)
    for j in range(64):
        eh[j, j * 64:(j + 1) * 64] = 1.0
    ew = np.zeros((64, 128), dtype=NPBF)
    idx = np.arange(64)
    ew[idx, idx] = 1.0
    ew[idx, 64 + idx] = 1.0

    in_maps = []
    for core in range(8):
        b, h = divmod(core, 4)
        wq = qkv_w[D * h:D * (h + 1)] * np.float32(0.125)       # (64, 256)
        wk = qkv_w[C + D * h:C + D * (h + 1)]
        wv = qkv_w[2 * C + D * h:2 * C + D * (h + 1)]
        wa = np.concatenate([wq.T, wk.T], axis=1).reshape(2, 128, 128)
        wb = np.concatenate([wk.T, wq.T], axis=1).reshape(2, 128, 128)
        in_maps.append({
            "x": np.ascontiguousarray(x[b].reshape(2, 128, HW)).astype(NPBF),
            "wa": np.ascontiguousarray(wa).astype(NPBF),
            "wb": np.ascontiguousarray(wb).astype(NPBF),
            "wv": np.ascontiguousarray(wv.T.reshape(2, 128, D)).astype(NPBF),
            "hrel": hrel_t,
            "wrel": wrel_t,
            "eh": eh,
            "ew": ew,
            "onesv": np.ones((128, NMC), dtype=NPBF),
        })
    return in_maps


def _assemble(results):
    out = np.empty((B, C, H, W), dtype=np.float32)
    for core in range(8):
        b, h = divmod(core, 4)
        unn = np.asarray(results[core]["out"], dtype=np.float32)
        out[b, D * h:D * (h + 1)] = (unn[0:D] / unn[D]).reshape(D, H, W)
    return out


def kernel(x, qkv_w, height_rel, width_rel):
    nc = _get_program()
    in_maps = _make_in_maps(x, qkv_w, height_rel, width_rel)
    res = run_bass_kernel_spmd(nc, in_maps, list(range(8)))
    return _assemble(res.results)


if __name__ == "__main__":
    rng = np.random.default_rng(0)
    xs = rng.standard_normal((B, C, H, W), dtype=np.float32)
    ws = rng.standard_normal((768, C), dtype=np.float32) * C ** -0.5
    hr = rng.standard_normal((2 * H - 1, D), dtype=np.float32) * D ** -0.5
    wr = rng.standard_normal((2 * W - 1, D), dtype=np.float32) * D ** -0.5
    o = kernel(xs, ws, hr, wr)
    print(o.shape, o.dtype, float(np.abs(o).mean()))


# revision 39
# speedup vs baseline: 1.1847x; 1.1847x over previous
"""BottleneckAttn Trainium2 kernel (bf16 PE pipeline).

Full inputs -> full output. 8-way head-parallel sharding, one (batch, head)
pair per NeuronCore. Per core, a fused transposed-attention kernel:

  attn^T[m, n] = sum_d k[d,m] q'[d,n] + XH^T[h'(m), n] + XW^T[w'(m), n]
  P^T = exp(attn^T)            (no row-max subtraction; logits are O(10))
  unnorm^T[dv, n] = sum_m v[m, dv] P^T[m, n];  S[n] = sum_m P^T[m, n]

with q' = SCALE*q folded into the q projection weights and rel tables
pre-scaled by 1/SCALE (exact powers of two). XH is injected into the logits
through 64 extra contraction rows (E_h selector stacked under k in the
stationary operand); XW through a second accumulating matmul against the
constant [I|I] selector. The softmax denominator S comes out as a 65th row
of the AV matmul (ones column in v^T); the final division by S happens on
the host (0.1% of the FLOPs) so the device never switches activation
tables away from Exp.

All attention matmuls run in bf16 (fp32 PSUM accumulation): on TRN2 bf16
streams at the same 1 column/cycle as fp32r but enables the compiler's
fast-weight-load path, which dominated the fp32r version. The q'/k
projection uses two M=128 stationaries [wq'|wk] and [wk|wq'] so both q'
(from A) and k (from B) land on PSUM partitions 0:64, matching their SBUF
row destinations without cross-partition moves. v^T is padded to a
128-column stationary for the same reason (cols 65:128 multiply into PSUM
partitions that are never read). The v^T projection and the XH blocks not
needed by the first n-block are interleaved into the first attention
n-block's instruction stream, where the PE and DVE have slack.
"""

import sys

if "/opt/trn_rl_repo" not in sys.path:
    sys.path.insert(0, "/opt/trn_rl_repo")

import ml_dtypes
import numpy as np

import concourse.bass as bass
import concourse.tile as tile
from concourse import bacc, mybir
from concourse.bass_utils import run_bass_kernel_spmd

B, C, H, W = 2, 256, 64, 64
NH, D = 4, 64
HW = H * W   # 4096
NMC = 32     # m chunks of 128
NBQ = 4      # n blocks of 1024
FP32 = mybir.dt.float32
BF16 = mybir.dt.bfloat16
AF = mybir.ActivationFunctionType
NPBF = ml_dtypes.bfloat16

_prog = None


def _build():
    nc = bacc.Bacc("TRN2", target_bir_lowering=False, debug=False)

    x_d = nc.dram_tensor("x", [2, 128, HW], BF16, kind="ExternalInput").ap()
    wa_d = nc.dram_tensor("wa", [2, 128, 128], BF16, kind="ExternalInput").ap()
    wb_d = nc.dram_tensor("wb", [2, 128, 128], BF16, kind="ExternalInput").ap()
    wv_d = nc.dram_tensor("wv", [2, 128, D], BF16, kind="ExternalInput").ap()
    hrel_d = nc.dram_tensor("hrel", [64, 127], BF16, kind="ExternalInput").ap()
    wrel_d = nc.dram_tensor("wrel", [64, 127], BF16, kind="ExternalInput").ap()
    eh_d = nc.dram_tensor("eh", [64, HW], BF16, kind="ExternalInput").ap()
    ew_d = nc.dram_tensor("ew", [64, 128], BF16, kind="ExternalInput").ap()
    onesv_d = nc.dram_tensor("onesv", [128, NMC], BF16, kind="ExternalInput").ap()
    out_d = nc.dram_tensor("out", [D + 1, HW], FP32, kind="ExternalOutput").ap()

    with tile.TileContext(nc) as tc:
        with (
            tc.tile_pool(name="const", bufs=1) as constp,
            tc.tile_pool(name="big", bufs=1) as bigp,
            tc.tile_pool(name="ptp", bufs=4) as ptp,
            tc.tile_pool(name="unn", bufs=2) as unnp,
            tc.tile_pool(name="ps", bufs=2, space="PSUM") as psp,
            tc.tile_pool(name="av_ps", bufs=1, space="PSUM") as avp,
        ):
            # ---------------- input tiles ----------------
            x_sb = bigp.tile([128, 2, HW], BF16)       # x[b]: (c, n), c = t*128+p
            wa_sb = constp.tile([128, 2, 128], BF16)   # [wq' | wk]
            wb_sb = constp.tile([128, 2, 128], BF16)   # [wk | wq']
            wv_sb = constp.tile([128, 2, D], BF16)
            hrel_sb = constp.tile([64, 127], BF16)     # height_rel^T / SCALE
            wrel_sb = constp.tile([64, 127], BF16)
            ew_sb = constp.tile([64, 128], BF16)       # [I64 | I64]
            # mm1 stationary: rows 0:64 = k, rows 64:128 = E_h; free = (mc, i)
            keh = bigp.tile([128, NMC, 128], BF16)
            # mm1 moving: rows 0:64 = q', rows 64:128 = XH^T; free = (h, w)
            rhs1 = bigp.tile([128, 64, 64], BF16)
            xw_t = bigp.tile([64, 64, 64], BF16)       # XW^T, free = (g, w)
            v_t = bigp.tile([128, NMC, 128], BF16)     # v^T + ones col + 63 junk

            for t in range(2):
                nc.sync.dma_start(out=wa_sb[:, t, :], in_=wa_d[t])
                nc.sync.dma_start(out=wb_sb[:, t, :], in_=wb_d[t])
                nc.sync.dma_start(out=wv_sb[:, t, :], in_=wv_d[t])
            nc.sync.dma_start(out=hrel_sb[:, :], in_=hrel_d[:, :])
            nc.sync.dma_start(out=wrel_sb[:, :], in_=wrel_d[:, :])
            for cb in range(8):
                xsl = slice(cb * 512, (cb + 1) * 512)
                for t in range(2):
                    nc.sync.dma_start(out=x_sb[:, t, xsl], in_=x_d[t, :, xsl])
            nc.sync.dma_start(out=keh[64:128, :, :], in_=eh_d[:, :])
            nc.sync.dma_start(out=ew_sb[:, :], in_=ew_d[:, :])
            nc.sync.dma_start(out=v_t[:, :, D], in_=onesv_d[:, :])

            # ---------------- phase 1: q', k projections ----------------
            # A = [wq'|wk], B = [wk|wq']: both q' (from A) and k (from B)
            # come out on PSUM partitions 0:64.
            # pre-phase psum tiles alternate between the "sm" and (otherwise
            # idle until phase 5) "pp"/"av" rings for 4-deep pipelining
            pre_n = 0

            def pre_ps(shape, name):
                nonlocal pre_n
                pre_n += 1
                if pre_n % 2 == 0:
                    return psp.tile(shape, FP32, name=name, tag="sm")
                elif pre_n % 4 == 1:
                    return psp.tile(shape, FP32, name=name, tag="pp")
                else:
                    return avp.tile(shape, FP32, name=name, tag="av")

            cp_n = 0

            def copy_alt(dst, src):
                # alternate copies between DVE and ACT (both idle pre-ph5)
                nonlocal cp_n
                cp_n += 1
                if cp_n % 2 == 0:
                    nc.scalar.copy(dst, src)
                else:
                    nc.vector.tensor_copy(dst, src)

            def vt_pair(mc):
                # v^T chunks mc, mc+1: stationary = x chunk (FWL), out = m
                psv = pre_ps([128, 2, D], "psv")
                for half in range(2):
                    msl = slice((mc + half) * 128, (mc + half + 1) * 128)
                    for t in range(2):
                        nc.tensor.matmul(
                            psv[:, half, :], x_sb[:, t, msl], wv_sb[:, t, :],
                            start=(t == 0), stop=(t == 1),
                        )
                copy_alt(v_t[:, mc:mc + 2, 0:D], psv[:, :, :])

            def xh_pair(r, on_act, in_ph5=False):
                # XH^T blocks r, r+1 -> rhs1[64:128, r:r+2, :]
                # (in phase 5 the pp/av psum rings are live: use "sm" only)
                if in_ph5:
                    psh = psp.tile([128, 2, 64], FP32, name="psh", tag="sm")
                else:
                    psh = pre_ps([128, 2, 64], "psh")
                for half in range(2):
                    nc.tensor.matmul(
                        psh[64:128, half, :],
                        hrel_sb[:, 63 - r - half:127 - r - half],
                        rhs1[0:64, r + half, :], start=True, stop=True,
                    )
                if on_act:
                    nc.scalar.copy(rhs1[64:128, r:r + 2, :], psh[64:128, :, :])
                else:
                    nc.vector.tensor_copy(
                        rhs1[64:128, r:r + 2, :], psh[64:128, :, :]
                    )

            for nb in range(8):
                sl = slice(nb * 512, (nb + 1) * 512)
                psa = pre_ps([128, 512], "psa")
                for t in range(2):
                    nc.tensor.matmul(
                        psa[:, :], wa_sb[:, t, :], x_sb[:, t, sl],
                        start=(t == 0), stop=(t == 1),
                    )
                nc.vector.tensor_copy(
                    rhs1[0:64, 8 * nb:8 * (nb + 1), :], psa[0:64, :]
                )
                psb = pre_ps([128, 512], "psb")
                for t in range(2):
                    nc.tensor.matmul(
                        psb[:, :], wb_sb[:, t, :], x_sb[:, t, sl],
                        start=(t == 0), stop=(t == 1),
                    )
                nc.scalar.copy(
                    keh[0:64, 4 * nb:4 * (nb + 1), :], psb[0:64, :]
                )

            # ---------- phase 4: XW^T, with v^T chunks interleaved -------
            for r in range(0, 64, 2):
                psw = pre_ps([64, 2, 64], "psw")
                for half in range(2):
                    nc.tensor.matmul(
                        psw[:, half, :],
                        wrel_sb[:, 63 - r - half:127 - r - half],
                        rhs1[0:64, :, r + half], start=True, stop=True,
                    )
                copy_alt(
                    xw_t[:, :, r:r + 2], psw[:, :, :].transpose([0, 2, 1])
                )
                if r % 8 == 2:
                    vt_pair((r // 8) * 4)
                    vt_pair((r // 8) * 4 + 2)

            # ---------------- phase 3a: XH blocks for first n block ------
            for r in range(0, 16, 2):
                xh_pair(r, on_act=(r % 4 == 0))

            # ---------------- phase 5: attention main loop ----------------
            # XH blocks for n block nbq+1 are interleaved into block nbq's
            # stream, one pair per 4 steps (PE and DVE have slack there).
            SKEW = 3
            for nbq in range(NBQ):
                av = avp.tile([128, 1024], FP32, name="av", tag="av")
                pts = {}
                for mc in range(NMC + SKEW):
                    if mc < NMC:
                        pp = psp.tile([128, 1024], FP32, name="pp", tag="pp")
                        for i in range(2):
                            nc.tensor.matmul(
                                pp[:, 512 * i:512 * (i + 1)],
                                keh[:, mc, :],
                                rhs1[:, 16 * nbq + 8 * i:16 * nbq + 8 * (i + 1), :],
                                start=True, stop=False,
                            )
                    if mc >= SKEW:
                        j = mc - SKEW
                        ptj = pts.pop(j)
                        for i in range(2):
                            nc.tensor.matmul(
                                av[:, 512 * i:512 * (i + 1)], v_t[:, j, :],
                                ptj[:, 512 * i:512 * (i + 1)],
                                start=(j == 0), stop=(j == NMC - 1),
                            )
                    if mc < NMC:
                        for i in range(2):
                            nc.tensor.matmul(
                                pp[:, 512 * i:512 * (i + 1)], ew_sb[:, :],
                                xw_t[:, 16 * nbq + 8 * i:16 * nbq + 8 * (i + 1), :],
                                start=False, stop=True,
                            )
                        pt = ptp.tile([128, 1024], BF16, name="pt")
                        nc.scalar.activation(pt[:, :], pp[:, :], AF.Exp)
                        pts[mc] = pt
                        if nbq < 3 and mc % 4 == 1:
                            xh_pair(16 * (nbq + 1) + (mc // 4) * 2,
                                    on_act=False, in_ph5=True)
                # unnormalized out^T rows 0:64 + S row; stage through SBUF
                for i in range(2):
                    unn = unnp.tile([65, 512], FP32, name="unn")
                    nc.vector.tensor_copy(
                        unn[:, :], av[0:65, 512 * i:512 * (i + 1)]
                    )
                    nc.sync.dma_start(
                        out=out_d[:, nbq * 1024 + 512 * i:
                                  nbq * 1024 + 512 * (i + 1)],
                        in_=unn[:, :],
                    )

    nc.finalize()
    return nc


def _get_program():
    global _prog
    if _prog is None:
        _prog = _build()
    return _prog


def _make_in_maps(x, qkv_w, height_rel, width_rel):
    x = np.ascontiguousarray(np.asarray(x, dtype=np.float32))
    qkv_w = np.ascontiguousarray(np.asarray(qkv_w, dtype=np.float32))
    height_rel = np.asarray(height_rel, dtype=np.float32)
    width_rel = np.asarray(width_rel, dtype=np.float32)

    # exact power-of-two rescale: q' = q/8 folded into wq, rel tables * 8
    hrel_t = np.ascontiguousarray((height_rel * np.float32(8.0)).T).astype(NPBF)
    wrel_t = np.ascontiguousarray((width_rel * np.float32(8.0)).T).astype(NPBF)

    eh = np.zeros((64, HW), dtype=NPBF)
    for j in range(64):
        eh[j, j * 64:(j + 1) * 64] = 1.0
    ew = np.zeros((64, 128), dtype=NPBF)
    idx = np.arange(64)
    ew[idx, idx] = 1.0
    ew[idx, 64 + idx] = 1.0

    in_maps = []
    for core in range(8):
        b, h = divmod(core, 4)
        wq = qkv_w[D * h:D * (h + 1)] * np.float32(0.125)       # (64, 256)
        wk = qkv_w[C + D * h:C + D * (h + 1)]
        wv = qkv_w[2 * C + D * h:2 * C + D * (h + 1)]
        wa = np.concatenate([wq.T, wk.T], axis=1).reshape(2, 128, 128)
        wb = np.concatenate([wk.T, wq.T], axis=1).reshape(2, 128, 128)
        in_maps.append({
            "x": np.ascontiguousarray(x[b].reshape(2, 128, HW)).astype(NPBF),
            "wa": np.ascontiguousarray(wa).astype(NPBF),
            "wb": np.ascontiguousarray(wb).astype(NPBF),
            "wv": np.ascontiguousarray(wv.T.reshape(2, 128, D)).astype(NPBF),
            "hrel": hrel_t,
            "wrel": wrel_t,
            "eh": eh,
            "ew": ew,
            "onesv": np.ones((128, NMC), dtype=NPBF),
        })
    return in_maps


def _assemble(results):
    out = np.empty((B, C, H, W), dtype=np.float32)
    for core in range(8):
        b, h = divmod(core, 4)
        unn = np.asarray(results[core]["out"], dtype=np.float32)
        out[b, D * h:D * (h + 1)] = (unn[0:D] / unn[D]).reshape(D, H, W)
    return out


def kernel(x, qkv_w, height_rel, width_rel):
    nc = _get_program()
    in_maps = _make_in_maps(x, qkv_w, height_rel, width_rel)
    res = run_bass_kernel_spmd(nc, in_maps, list(range(8)))
    return _assemble(res.results)


if __name__ == "__main__":
    rng = np.random.default_rng(0)
    xs = rng.standard_normal((B, C, H, W), dtype=np.float32)
    ws = rng.standard_normal((768, C), dtype=np.float32) * C ** -0.5
    hr = rng.standard_normal((2 * H - 1, D), dtype=np.float32) * D ** -0.5
    wr = rng.standard_normal((2 * W - 1, D), dtype=np.float32) * D ** -0.5
    o = kernel(xs, ws, hr, wr)
    print(o.shape, o.dtype, float(np.abs(o).mean()))
